# revision 31
# baseline (speedup 1.0000x reference)
"""GAT (2-layer, 8-head) Bass kernel for 8 Trainium2 NeuronCores.

Strategy (edge-parallel, dst-sharded), transfer-optimized:
  A call's wall-clock is dominated by host->device transfer over the axon
  tunnel (~70 MB/s, per-array fixed overhead); device execution of the
  whole edge phase measures only ~5 ms. So all per-core inputs are packed
  into ONE int16 tensor (~1.33 MB/core):

  - x ships as int8, quantized per node (scale = max|x_row|/127, f16
    scales shipped alongside), pre-transposed into 128x128 lhsT tiles and
    packed two tiles per 128 rows. On-device: int8 -> f16 copy, f16 PE
    matmul, and the per-node scale is applied by the PSUM->SBUF copy
    (tensor_scalar mult with a per-partition scalar).
  - Weights ship as float16 (W1, W2, W@A_blockdiag fused on host, Wout);
    bias rows and an iota row ship as single 128-wide rows and are
    broadcast across partitions on-device with a rank-1 PE matmul.
  - The dma_gather index table ships 16-partition-wrapped ONLY (the
    128-partition replication the ISA wants is rebuilt on-device via 8
    DMAs).
  - dstoff (edge -> destination row-in-window) ships as uint8 (255 = pad).
  - The output ships back as float16.

  Device algorithm (f32 internally):
  - Nodes split into 8 slices of 6250; core c owns slice c (processes all
    edges whose dst is in slice c).
  - Each core builds its slice of a node record table
    [h (128) | a_src (8) | a_dst (8) | pad] = 192 f32/row, AllGather
    replicates the full table to every core.
  - Edges are dst-sorted and bucketed into fixed 128-row destination
    windows; per 128-edge tile a one-hot (edge x window-row) matrix is
    built with one is_equal op and a PE matmul accumulates messages into a
    PSUM window. Per-edge softmax weight w = exp(leaky_relu(as[src] +
    ad[dst])); as comes with the gathered 768B src record; ad via a PE
    matmul of the TRANSPOSED one-hot against the local records (no second
    gather, no alpha-table upload). Denominator = window-accumulated w;
    divide + bias + relu at node level; repeat for layer 2; output
    projection.

  The 50176-row table is split in halves for int16 gather indices; edges
  are processed in two passes by src-half. The window/tile schedule is
  computed on the host from edge_index and baked into the program.
"""

import sys
import os

for _p in ("/opt/trn_rl_repo", "/root/.axon_site/_ro/trn_rl_repo"):
    if os.path.isdir(_p) and _p not in sys.path:
        sys.path.insert(0, _p)

import numpy as np

NEG_SLOPE = 0.2
WW = 128      # window rows = one 128-node block (partition-aligned)


def full_cfg():
    return dict(cores=8, n=50000, tb=49, cb=8, in_ch=128, hc=128,
                heads=8, hid=16, ncls=10)


def derive(cfg):
    d = dict(cfg)
    d["slice"] = d["n"] // d["cores"]
    d["slice_pad"] = d["tb"] * 128
    d["table_rows"] = d["cores"] * d["slice_pad"]
    d["half_rows"] = d["table_rows"] // 2
    d["trw"] = 192                     # table row width (f32)
    d["mw"] = d["hc"] + d["heads"]     # message width: h|w
    d["arw"] = 64                      # alpha table row width
    d["chunk"] = 128 * d["cb"]
    d["nwin"] = d["tb"]
    assert d["slice"] <= d["slice_pad"]
    return d


def pack_layout(c, ntot):
    """Row offsets into the per-core [R, 128] int16 pack tensor."""
    K = -(-(ntot * 8) // 128)      # columns (of 128 i16) for idx tables
    Q = -(-ntot // 256)            # 128-row groups for the u8 dstoff table
    ngx = -(-c["tb"] // 2)         # int8 xT tile pairs
    off = {}
    r = 0
    off["xT8"] = r; r += ngx * 128   # i8: tile 2g in bytes 0:128, 2g+1 in 128:256
    off["sS"] = r; r += 128          # f16 [128, tb] per-node dequant scales
    off["gidx"] = r; r += 16 * K
    off["dstoff"] = r; r += Q * 128  # u8 [128, ntot] (255 = pad slot)
    off["W1"] = r; r += 128
    off["W2"] = r; r += 128
    off["misc"] = r; r += 128        # f16 cols: 0:16 WA1 | 16:32 WA2 | 32:42 Wout
    off["rows4"] = r; r += 4         # f16 rows: iota | b1 | b2 | bout
    off["R"] = r
    off["K"] = K
    off["Q"] = Q
    return off


# ---------------------------------------------------------------- host prep

def _table_row(nid, c):
    nl = nid % c["slice"]
    return (nid // c["slice"]) * c["slice_pad"] + (nl % 128) * c["tb"] + nl // 128


def _acc_row(nl, c):
    return (nl % 128) * c["tb"] + nl // 128


def host_prep(x, edge_index, c):
    """Build per-core edge maps + the shared (max-over-cores) window schedule.

    Returns (edge_maps, sched): edge_maps[core] has gidx [16, ntot*8] i16,
    aidx [16, ntot*8] i16, dstoff [128, ntot] f16.
    """
    n, cores = c["n"], c["cores"]
    sl, tb, cb = c["slice"], c["tb"], c["cb"]
    nwin = c["nwin"]
    src = np.concatenate([np.asarray(edge_index[0], np.int64),
                          np.arange(n, dtype=np.int64)])
    dst = np.concatenate([np.asarray(edge_index[1], np.int64),
                          np.arange(n, dtype=np.int64)])
    trow = _table_row(src, c)
    half = (trow >= c["half_rows"]).astype(np.int64)
    owner = dst // sl
    dloc = dst % sl
    win = dloc // WW

    # counts per (core, half, window) in one bincount
    key = (owner * 2 + half) * nwin + win
    counts = np.bincount(key, minlength=cores * 2 * nwin).reshape(cores, 2, nwin)
    tpw = -(-counts.max(axis=0) // 128)          # [2, nwin] tiles per window
    ntiles = tpw.sum(axis=1)
    for h in (0, 1):
        padt = (-int(ntiles[h])) % cb
        if padt:
            nz = np.nonzero(tpw[h])[0]
            wlast = int(nz[-1]) if len(nz) else 0
            tpw[h, wlast] += padt
            ntiles[h] += padt
    sched = dict(tpw=tpw, ntiles=[int(ntiles[0]), int(ntiles[1])])

    ntot = int(ntiles.sum())
    cap = ntot * 128
    # tile start (global, across both halves) for each (half, window)
    tstart = np.zeros((2, nwin), np.int64)
    tstart[0] = np.cumsum(tpw[0]) - tpw[0]
    tstart[1] = int(ntiles[0]) + np.cumsum(tpw[1]) - tpw[1]

    maps = []
    for core in range(cores):
        m = owner == core
        tr_c = trow[m]
        dl_c = dloc[m]
        hf_c = half[m]
        # order edges by (half, dloc) -> grouped per (half, window), windows
        # ascending, and dloc ascending inside each window
        order = np.lexsort((dl_c, hf_c))
        tr_c, dl_c, hf_c = tr_c[order], dl_c[order], hf_c[order]
        wn_c = dl_c // WW

        # slot of each edge: base of its (half, window) bucket + rank inside
        bkey = hf_c * nwin + wn_c
        bcnt = np.bincount(bkey, minlength=2 * nwin)
        bstart = np.cumsum(bcnt) - bcnt             # start idx in sorted order
        rank = np.arange(len(bkey)) - bstart[bkey]
        base = (tstart.reshape(-1)[bkey]) * 128
        slots = base + rank
        assert np.all(rank < tpw.reshape(-1)[bkey] * 128)

        srcrow = np.zeros(cap, np.int64)            # pads: row 0
        dstoff = np.full(cap, 255, np.uint8)        # pads: no one-hot match
        srcrow[slots] = tr_c - hf_c * c["half_rows"]
        dstoff[slots] = (dl_c % 128).astype(np.uint8)

        def wrap16(vals):
            nq = ntot // cb
            v = vals.reshape(nq, cb * 128)
            w16 = np.zeros((nq, 16, cb * 8), np.int16)
            k = np.arange(cb * 128)
            w16[:, k % 16, k // 16] = v
            return w16.transpose(1, 0, 2).reshape(16, nq * cb * 8)

        maps.append(dict(
            gidx=wrap16(srcrow.astype(np.int16)),
            dstoff=np.ascontiguousarray(dstoff.reshape(ntot, 128).T),
        ))
    return maps, sched


def host_pack(x, edge_maps, sched, W1, a_src1, a_dst1, b1, W2, a_src2,
              a_dst2, b2, Wout, bout, c):
    """Assemble the per-core [R, 128] int16 pack tensors."""
    heads, hid, hc, tb = c["heads"], c["hid"], c["hc"], c["tb"]
    sl, sp = c["slice"], c["slice_pad"]
    ntot = int(sched["ntiles"][0] + sched["ntiles"][1])
    lay = pack_layout(c, ntot)

    def blockdiag(a):
        A = np.zeros((hc, heads), np.float32)
        for h in range(heads):
            A[h * hid:(h + 1) * hid, h] = a[h]
        return A

    W1 = np.asarray(W1, np.float32)
    W2 = np.asarray(W2, np.float32)
    WA1 = np.concatenate([W1 @ blockdiag(np.asarray(a_src1, np.float32)),
                          W1 @ blockdiag(np.asarray(a_dst1, np.float32))], axis=1)
    WA2 = np.concatenate([W2 @ blockdiag(np.asarray(a_src2, np.float32)),
                          W2 @ blockdiag(np.asarray(a_dst2, np.float32))], axis=1)
    misc = np.zeros((128, 128), np.float16)
    misc[:, 0:16] = WA1.astype(np.float16)
    misc[:, 16:32] = WA2.astype(np.float16)
    misc[:, 32:42] = np.asarray(Wout, np.float32).astype(np.float16)
    rows4 = np.zeros((4, 128), np.float16)
    rows4[0] = np.arange(128, dtype=np.float16)
    rows4[1] = np.asarray(b1, np.float32).astype(np.float16)
    rows4[2] = np.asarray(b2, np.float32).astype(np.float16)
    rows4[3, 0:c["ncls"]] = np.asarray(bout, np.float32).astype(np.float16)

    x = np.asarray(x, np.float32)
    ngx = -(-tb // 2)
    packs = []
    for core in range(c["cores"]):
        em = edge_maps[core]
        p = np.zeros((lay["R"], 128), np.int16)
        xs = np.zeros((sp, c["in_ch"]), np.float32)
        xs[:sl] = x[core * sl:(core + 1) * sl]
        s = np.maximum(np.abs(xs).max(axis=1) / 127.0, 1e-4).astype(np.float16)
        xq = np.round(xs / s[:, None].astype(np.float32)).clip(-127, 127)
        # per-tile transposed lhsT blocks [128 in_ch, 128 nodes], int8
        xT = np.ascontiguousarray(
            xq.astype(np.int8).reshape(tb, 128, c["in_ch"]).transpose(0, 2, 1))
        blocks = np.zeros((ngx, 128, 256), np.int8)
        blocks[:, :, 0:128] = xT[0::2]
        blocks[: tb // 2, :, 128:256] = xT[1::2]
        p[lay["xT8"]:lay["xT8"] + ngx * 128] = \
            blocks.reshape(ngx * 128, 256).view(np.int16)
        srow = np.zeros((128, 128), np.float16)
        srow[:, :tb] = s.reshape(tb, 128).T
        p[lay["sS"]:lay["sS"] + 128] = srow.view(np.int16)

        K = lay["K"]
        gi = np.zeros((16, K * 128), np.int16)
        gi[:, :ntot * 8] = em["gidx"]
        p[lay["gidx"]:lay["gidx"] + 16 * K] = gi.reshape(16 * K, 128)

        Q = lay["Q"]
        doff = np.full((128, Q * 256), 255, np.uint8)
        doff[:, :ntot] = em["dstoff"]
        p[lay["dstoff"]:lay["dstoff"] + Q * 128] = np.ascontiguousarray(
            doff.reshape(128, Q, 256).transpose(1, 0, 2)
        ).reshape(Q * 128, 256).view(np.int16)

        p[lay["W1"]:lay["W1"] + 128] = W1.astype(np.float16).view(np.int16)
        p[lay["W2"]:lay["W2"] + 128] = W2.astype(np.float16).view(np.int16)
        p[lay["misc"]:lay["misc"] + 128] = misc.view(np.int16)
        p[lay["rows4"]:lay["rows4"] + 4] = rows4.view(np.int16)
        packs.append(p)
    return packs


def host_post(results, c):
    n = c["n"]
    out = np.zeros((n, c["ncls"]), np.float32)
    rows = _acc_row(np.arange(c["slice"]), c)
    for core in range(c["cores"]):
        res = np.asarray(results[core]["out"], np.float32)
        out[core * c["slice"]:(core + 1) * c["slice"]] = res[rows]
    return out


# ---------------------------------------------------------------- device build

def build_nc(c, sched, run_edges=True):
    from concourse import bass, mybir, bacc, tile
    from concourse.masks import make_identity

    f32 = mybir.dt.float32
    f16 = mybir.dt.float16
    i16 = mybir.dt.int16
    i8 = mybir.dt.int8
    u8 = mybir.dt.uint8
    Alu = mybir.AluOpType
    Act = mybir.ActivationFunctionType

    nc = bacc.Bacc("TRN2", target_bir_lowering=False, debug=False,
                   num_devices=c["cores"])
    cores = list(range(c["cores"]))

    tb, cb = c["tb"], c["cb"]
    hc, heads, ncls = c["hc"], c["heads"], c["ncls"]
    trw, mw = c["trw"], c["mw"]
    sp, nwin = c["slice_pad"], c["nwin"]
    tpw, ntiles = sched["tpw"], sched["ntiles"]
    ntot = int(ntiles[0] + ntiles[1])
    lay = pack_layout(c, ntot)
    K, Q = lay["K"], lay["Q"]

    # ---- I/O
    pack = nc.dram_tensor("pack", [lay["R"], 128], i16, kind="ExternalInput")
    out = nc.dram_tensor("out", [sp, ncls], f16, kind="ExternalOutput")

    # ---- internal DRAM
    bounce1 = nc.dram_tensor("bounce1", [sp, trw], f32)
    bounce2 = nc.dram_tensor("bounce2", [sp, trw], f32)
    tspace = "Shared" if c["cores"] > 4 else "Local"
    table1 = nc.dram_tensor("table1", [c["table_rows"], trw], f32, addr_space=tspace)
    table2 = nc.dram_tensor("table2", [c["table_rows"], trw], f32, addr_space=tspace)

    with tile.TileContext(nc) as tc:
        with (
            tc.tile_pool(name="const", bufs=1) as constp,
            tc.tile_pool(name="rec", bufs=1) as recp,
            tc.tile_pool(name="big", bufs=2) as bigp,
            tc.tile_pool(name="accs", bufs=1) as accsp,
            tc.tile_pool(name="small", bufs=2) as smallp,
            tc.tile_pool(name="work", bufs=2) as workp,
            tc.tile_pool(name="oh", bufs=2 * c["cb"]) as ohp,
            tc.tile_pool(name="oht", bufs=3) as ohtp,
            tc.tile_pool(name="psA", bufs=1, space="PSUM") as psA,
            tc.tile_pool(name="psB", bufs=1, space="PSUM") as psB,
            tc.tile_pool(name="psD", bufs=1, space="PSUM") as psD,
            tc.tile_pool(name="psW", bufs=2, space="PSUM") as psW,
            tc.tile_pool(name="psT2", bufs=2, space="PSUM") as psT2,
            tc.tile_pool(name="psAD", bufs=1, space="PSUM") as psAD,
        ):
            # ---------------- constants ----------------
            ident = constp.tile([128, 128], f32, tag="ident")
            make_identity(nc, ident[:])

            W1s = constp.tile([128, hc], f16, tag="W1s")
            nc.sync.dma_start(W1s[:], pack[lay["W1"]:lay["W1"] + 128, :].bitcast(f16))
            W2s = constp.tile([128, hc], f16, tag="W2s")
            nc.sync.dma_start(W2s[:], pack[lay["W2"]:lay["W2"] + 128, :].bitcast(f16))
            miscS = constp.tile([128, 128], f16, tag="miscS")
            nc.sync.dma_start(miscS[:], pack[lay["misc"]:lay["misc"] + 128, :].bitcast(f16))
            rowsS = []
            for i in range(4):
                rS = constp.tile([1, 128], f16, tag=f"row{i}S")
                nc.sync.dma_start(
                    rS[:], pack[lay["rows4"] + i:lay["rows4"] + i + 1, :].bitcast(f16))
                rowsS.append(rS)
            onesr = constp.tile([1, 128], f16, tag="onesr")
            nc.vector.memset(onesr[:], 1.0)

            # broadcast single rows across partitions: rank-1 PE matmul
            def bcast_row(row_ap, width, tag):
                ps = psA.tile([128, 128], f32, tag="psT")
                nc.tensor.matmul(out=ps[:, 0:width], lhsT=onesr[:],
                                 rhs=row_ap, start=True, stop=True)
                t = constp.tile([128, width], f32, tag=tag)
                nc.any.tensor_copy(out=t[:], in_=ps[:, 0:width])
                return t

            iotaS = bcast_row(rowsS[0][:], 128, "iotaS")
            b1s = bcast_row(rowsS[1][:], hc, "b1s")
            b2s = bcast_row(rowsS[2][:], hc, "b2s")
            bouts = bcast_row(rowsS[3][:, 0:ncls], ncls, "bouts")

            # gather index tables: load [16, K*128] and replicate to 128 parts
            gidxS = constp.tile([128, K * 128], i16, tag="gidxS")
            gsrc = pack[lay["gidx"]:lay["gidx"] + 16 * K, :].rearrange(
                "(p k) w -> p (k w)", p=16)
            for k in range(8):
                nc.sync.dma_start(gidxS[16 * k:16 * (k + 1), :], gsrc)

            dstoff8 = constp.tile([128, Q * 256], u8, tag="dstoff8")
            for q in range(Q):
                nc.sync.dma_start(
                    dstoff8[:, q * 256:(q + 1) * 256],
                    pack[lay["dstoff"] + q * 128:lay["dstoff"] + (q + 1) * 128, :]
                    .bitcast(u8))
            dstoffS = constp.tile([128, ntot], f32, tag="dstoffS")
            nc.vector.tensor_copy(out=dstoffS[:], in_=dstoff8[:, 0:ntot])

            sS16 = constp.tile([128, 128], f16, tag="sS16")
            nc.sync.dma_start(sS16[:], pack[lay["sS"]:lay["sS"] + 128, :].bitcast(f16))
            sF = constp.tile([128, tb], f32, tag="sF")
            nc.vector.tensor_copy(out=sF[:], in_=sS16[:, 0:tb])

            accS = accsp.tile([128, tb, mw], f32, tag="accS")

            # ---------------- record-slice build ----------------
            def build_records(get_lhsT, W, WA, rec, scale=None):
                nc.vector.memset(rec[:], 0.0)
                for t in range(tb):
                    lt = get_lhsT(t)
                    h_p = psB.tile([128, hc], f32, tag="psH")
                    nc.tensor.matmul(out=h_p[:], lhsT=lt, rhs=W, start=True, stop=True)
                    a_p = psD.tile([128, 2 * heads], f32, tag="psAS")
                    nc.tensor.matmul(out=a_p[:], lhsT=lt, rhs=WA, start=True, stop=True)
                    if scale is None:
                        nc.any.tensor_copy(out=rec[:, t, 0:hc], in_=h_p[:])
                        nc.any.tensor_copy(out=rec[:, t, hc:hc + 2 * heads], in_=a_p[:])
                    else:
                        nc.vector.tensor_scalar(
                            out=rec[:, t, 0:hc], in0=h_p[:],
                            scalar1=scale[:, t:t + 1], scalar2=None, op0=Alu.mult)
                        nc.vector.tensor_scalar(
                            out=rec[:, t, hc:hc + 2 * heads], in0=a_p[:],
                            scalar1=scale[:, t:t + 1], scalar2=None, op0=Alu.mult)

            def publish(rec, bounce, table):
                nc.sync.dma_start(
                    bounce[:].rearrange("(p t) w -> p t w", p=128), rec[:]
                )
                nc.gpsimd.collective_compute(
                    "AllGather", mybir.AluOpType.bypass,
                    replica_groups=[cores], ins=[bounce[:]], outs=[table[:]],
                )

            # ---------------- edge phase ----------------
            def edge_phase(table, rec):
                nc.vector.memset(accS[:], 0.0)
                if not run_edges:
                    return
                tile_base = 0
                for h in (0, 1):
                    tab_h = table[h * c["half_rows"]:(h + 1) * c["half_rows"], :]
                    nt_h = int(ntiles[h])
                    nq = nt_h // cb
                    wins = []
                    twin = []                     # tile (within half) -> window
                    t0 = 0
                    for w in range(nwin):
                        tcnt = int(tpw[h, w])
                        if tcnt:
                            wins.append((w, t0, tcnt))
                            twin.extend([w] * tcnt)
                            t0 += tcnt
                    assert t0 == nt_h
                    widx = 0
                    psw = None
                    for q in range(nq):
                        grec = bigp.tile([128, cb, trw], f32, tag="grec")
                        ccol = (tile_base + q * cb) * 8
                        nc.gpsimd.dma_gather(
                            out_ap=grec[:], in_ap=tab_h,
                            idxs_ap=gidxS[:, ccol:ccol + cb * 8],
                            num_idxs=cb * 128, num_idxs_reg=cb * 128,
                            elem_size=trw,
                        )
                        # one-hots for the chunk's tiles + per-edge a_dst via
                        # transposed-one-hot matmul against the local records
                        ohs = []
                        ps_ad = psAD.tile([128, cb, heads], f32, tag="psad")
                        for b in range(cb):
                            g_h = q * cb + b
                            gg = tile_base + g_h
                            oh = ohp.tile([128, 128], f32, tag="oh")
                            nc.vector.tensor_scalar(
                                out=oh[:], in0=iotaS[:],
                                scalar1=dstoffS[:, gg:gg + 1], scalar2=None,
                                op0=Alu.is_equal,
                            )
                            ohs.append(oh)
                            ohT_p = psT2.tile([128, 128], f32, tag="psoT")
                            nc.tensor.transpose(out=ohT_p[:], in_=oh[:],
                                                identity=ident[:])
                            ohTs = ohtp.tile([128, 128], f32, tag="ohT")
                            nc.any.tensor_copy(out=ohTs[:], in_=ohT_p[:])
                            wb = twin[g_h]
                            nc.tensor.matmul(
                                out=ps_ad[:, b, :], lhsT=ohTs[:],
                                rhs=rec[:, wb, hc + heads:hc + 2 * heads],
                                start=True, stop=True,
                            )
                        wv = smallp.tile([128, cb, heads], f32, tag="wv")
                        tmp = smallp.tile([128, cb, heads], f32, tag="tmp")
                        nc.vector.tensor_tensor(
                            out=wv[:], in0=grec[:, :, hc:hc + heads],
                            in1=ps_ad[:], op=Alu.add,
                        )
                        nc.vector.tensor_scalar(
                            out=tmp[:], in0=wv[:], scalar1=0.0,
                            scalar2=-(1.0 - NEG_SLOPE), op0=Alu.min, op1=Alu.mult,
                        )
                        nc.vector.tensor_tensor(
                            out=wv[:], in0=wv[:], in1=tmp[:], op=Alu.add,
                        )
                        nc.scalar.activation(out=wv[:], in_=wv[:], func=Act.Exp)
                        nc.vector.tensor_tensor(
                            out=grec[:, :, 0:hc].rearrange(
                                "p b (h d) -> p b h d", h=heads),
                            in0=grec[:, :, 0:hc].rearrange(
                                "p b (h d) -> p b h d", h=heads),
                            in1=wv[:].unsqueeze(-1).to_broadcast(
                                [128, cb, heads, c["hid"]]),
                            op=Alu.mult,
                        )
                        nc.vector.tensor_copy(
                            out=grec[:, :, hc:hc + heads], in_=wv[:]
                        )
                        for b in range(cb):
                            g_h = q * cb + b
                            w, t0w, tcnt = wins[widx]
                            if g_h == t0w:
                                psw = psW.tile([128, mw], f32, tag="psw")
                            nc.tensor.matmul(
                                out=psw[:], lhsT=ohs[b][:], rhs=grec[:, b, 0:mw],
                                start=g_h == t0w, stop=g_h == t0w + tcnt - 1,
                            )
                            if g_h == t0w + tcnt - 1:
                                nc.vector.tensor_tensor(
                                    out=accS[:, w, :], in0=accS[:, w, :],
                                    in1=psw[:], op=Alu.add,
                                )
                                widx += 1
                    tile_base += nt_h

            # ---------------- divide + bias + relu ----------------
            def finish_layer(bias, ytile):
                rcp = smallp.tile([128, tb, heads], f32, tag="rcp")
                nc.vector.tensor_scalar(
                    out=rcp[:], in0=accS[:, :, hc:hc + heads],
                    scalar1=1e-9, scalar2=None, op0=Alu.add,
                )
                nc.vector.reciprocal(out=rcp[:], in_=rcp[:])
                nc.vector.tensor_tensor(
                    out=ytile[:].rearrange("p t (h d) -> p t h d", h=heads),
                    in0=accS[:, :, 0:hc].rearrange("p t (h d) -> p t h d", h=heads),
                    in1=rcp[:].unsqueeze(-1).to_broadcast([128, tb, heads, c["hid"]]),
                    op=Alu.mult,
                )
                nc.vector.tensor_tensor(
                    out=ytile[:], in0=ytile[:],
                    in1=bias.unsqueeze(1).to_broadcast([128, tb, hc]),
                    op=Alu.add,
                )
                nc.vector.tensor_scalar(
                    out=ytile[:], in0=ytile[:], scalar1=0.0, scalar2=None,
                    op0=Alu.max,
                )

            # lhsT providers: layer 1 reads pre-transposed f16 tiles from the
            # pack; later layers transpose on-device and downcast to f16.
            def x_lhsT(t):
                g, hb = t // 2, t % 2
                xq8 = workp.tile([128, 128], i8, tag="xq8")
                r0 = lay["xT8"] + g * 128
                nc.sync.dma_start(
                    xq8[:], pack[r0:r0 + 128, 64 * hb:64 * (hb + 1)].bitcast(i8))
                xt = workp.tile([128, 128], f16, tag="xt")
                nc.vector.tensor_copy(out=xt[:], in_=xq8[:])
                return xt[:]

            def make_y_lhsT(ytile):
                def y_lhsT(t):
                    yT_p = psA.tile([128, 128], f32, tag="psT")
                    nc.tensor.transpose(out=yT_p[:], in_=ytile[:, t, :],
                                        identity=ident[:])
                    yTs = workp.tile([128, 128], f16, tag="xt")
                    nc.any.tensor_copy(out=yTs[:], in_=yT_p[:])
                    return yTs[:]
                return y_lhsT

            # ================ layer 1 ================
            rec1 = recp.tile([128, tb, trw], f32, tag="rec")
            build_records(x_lhsT, W1s[:], miscS[:, 0:16], rec1, scale=sF)
            publish(rec1, bounce1, table1)
            edge_phase(table1, rec1)
            y1 = recp.tile([128, tb, hc], f32, tag="y")
            finish_layer(b1s[:], y1)

            # ================ layer 2 ================
            rec2 = recp.tile([128, tb, trw], f32, tag="rec")
            build_records(make_y_lhsT(y1), W2s[:], miscS[:, 16:32], rec2)
            publish(rec2, bounce2, table2)
            edge_phase(table2, rec2)
            y2 = recp.tile([128, tb, hc], f32, tag="y")
            finish_layer(b2s[:], y2)

            # ================ output projection ================
            outt = recp.tile([128, tb, ncls], f16, tag="outt")
            y_lhsT2 = make_y_lhsT(y2)
            for t in range(tb):
                yTs = y_lhsT2(t)
                o_p = psD.tile([128, 2 * heads], f32, tag="psAS")
                nc.tensor.matmul(out=o_p[:, 0:ncls], lhsT=yTs,
                                 rhs=miscS[:, 32:32 + ncls],
                                 start=True, stop=True)
                nc.vector.tensor_tensor(
                    out=outt[:, t, :], in0=o_p[:, 0:ncls], in1=bouts[:],
                    op=Alu.add,
                )
            nc.sync.dma_start(
                out[:].rearrange("(p t) w -> p t w", p=128), outt[:]
            )

    nc.compile()
    return nc


# ---------------------------------------------------------------- entry point

_CACHE = {}


def prepare(inputs, c):
    """inputs dict -> (in_maps, sched)."""
    x = np.asarray(inputs["x"], np.float32)
    edge_index = np.asarray(inputs["edge_index"])
    edge_maps, sched = host_prep(x, edge_index, c)
    packs = host_pack(
        x, edge_maps, sched, inputs["W1"], inputs["a_src1"], inputs["a_dst1"],
        inputs["b1"], inputs["W2"], inputs["a_src2"], inputs["a_dst2"],
        inputs["b2"], inputs["Wout"], inputs["bout"], c)
    in_maps = [dict(pack=p) for p in packs]
    return in_maps, sched


def kernel(x, edge_index, W1, a_src1, a_dst1, b1, W2, a_src2, a_dst2, b2,
           Wout, bout):
    from concourse.bass_utils import run_bass_kernel_spmd

    c = derive(full_cfg())
    in_maps, sched = prepare(dict(
        x=x, edge_index=edge_index, W1=W1, a_src1=a_src1, a_dst1=a_dst1,
        b1=b1, W2=W2, a_src2=a_src2, a_dst2=a_dst2, b2=b2, Wout=Wout,
        bout=bout), c)
    key = ("full", sched["tpw"].tobytes())
    if key not in _CACHE:
        _CACHE[key] = build_nc(c, sched)
    nc = _CACHE[key]
    res = run_bass_kernel_spmd(nc, in_maps, list(range(c["cores"])))
    return host_post(res.results, c)


# revision 32
# speedup vs baseline: 1.1076x; 1.1076x over previous
"""GAT (2-layer, 8-head) Bass kernel for 8 Trainium2 NeuronCores.

Strategy (edge-parallel, dst-sharded), transfer-optimized:
  A call's wall-clock is dominated by host->device transfer over the axon
  tunnel (~70 MB/s, per-array fixed overhead); device execution of the
  whole edge phase measures only ~5 ms. So all per-core inputs are packed
  into ONE int16 tensor (~1.33 MB/core):

  - x ships as int8, quantized per node (scale = max|x_row|/127, f16
    scales shipped alongside), pre-transposed into 128x128 lhsT tiles and
    packed two tiles per 128 rows. On-device: int8 -> f16 copy, f16 PE
    matmul, and the per-node scale is applied by the PSUM->SBUF copy
    (tensor_scalar mult with a per-partition scalar).
  - Weights ship as float16 (W1, W2, W@A_blockdiag fused on host, Wout);
    bias rows and an iota row ship as single 128-wide rows and are
    broadcast across partitions on-device with a rank-1 PE matmul.
  - The dma_gather index table ships 16-partition-wrapped ONLY (the
    128-partition replication the ISA wants is rebuilt on-device via 8
    DMAs).
  - dstoff (edge -> destination row-in-window) ships as uint8 (255 = pad).
  - The output ships back as float16.

  Device algorithm (f32 internally):
  - Nodes split into 8 slices of 6250; core c owns slice c (processes all
    edges whose dst is in slice c).
  - Each core builds its slice of a node record table
    [h (128) | a_src (8) | a_dst (8) | pad] = 192 f32/row, AllGather
    replicates the full table to every core.
  - Edges are dst-sorted and bucketed into fixed 128-row destination
    windows; per 128-edge tile a one-hot (edge x window-row) matrix is
    built with one is_equal op and a PE matmul accumulates messages into a
    PSUM window. Per-edge softmax weight w = exp(leaky_relu(as[src] +
    ad[dst])); as comes with the gathered 768B src record; ad via a PE
    matmul of the TRANSPOSED one-hot against the local records (no second
    gather, no alpha-table upload). Denominator = window-accumulated w;
    divide + bias + relu at node level; repeat for layer 2; output
    projection.

  The 50176-row table is split in halves for int16 gather indices; edges
  are processed in two passes by src-half. The window/tile schedule is
  computed on the host from edge_index and baked into the program.
"""

import sys
import os

for _p in ("/opt/trn_rl_repo", "/root/.axon_site/_ro/trn_rl_repo"):
    if os.path.isdir(_p) and _p not in sys.path:
        sys.path.insert(0, _p)

import numpy as np

NEG_SLOPE = 0.2
WW = 128      # window rows = one 128-node block (partition-aligned)


def full_cfg():
    return dict(cores=8, n=50000, tb=49, cb=8, in_ch=128, hc=128,
                heads=8, hid=16, ncls=10)


def derive(cfg):
    d = dict(cfg)
    d["slice"] = d["n"] // d["cores"]
    d["slice_pad"] = d["tb"] * 128
    d["table_rows"] = d["cores"] * d["slice_pad"]
    d["half_rows"] = d["table_rows"] // 2
    d["trw"] = 192                     # table row width (f32)
    d["mw"] = d["hc"] + d["heads"]     # message width: h|w
    d["arw"] = 64                      # alpha table row width
    d["chunk"] = 128 * d["cb"]
    d["nwin"] = d["tb"]
    assert d["slice"] <= d["slice_pad"]
    return d


def pack_layout(c, ntot):
    """Row offsets into the per-core [R, 128] int16 pack tensor."""
    K = -(-(ntot * 8) // 128)      # columns (of 128 i16) for idx tables
    Q = -(-ntot // 256)            # 128-row groups for the u8 dstoff table
    ngx = -(-c["tb"] // 2)         # int8 xT tile pairs
    off = {}
    r = 0
    off["xT8"] = r; r += ngx * 128   # i8: tile 2g in bytes 0:128, 2g+1 in 128:256
    off["sS"] = r; r += 128          # f16 [128, tb] per-node dequant scales
    off["gidx"] = r; r += 16 * K
    off["dstoff"] = r; r += Q * 128  # u8 [128, ntot] (255 = pad slot)
    off["W1"] = r; r += 128
    off["W2"] = r; r += 128
    off["misc"] = r; r += 128        # f16 cols: 0:16 WA1 | 16:32 WA2 | 32:42 Wout
    off["rows4"] = r; r += 4         # f16 rows: iota | b1 | b2 | bout
    off["R"] = r
    off["K"] = K
    off["Q"] = Q
    return off


# ---------------------------------------------------------------- host prep

def _table_row(nid, c):
    nl = nid % c["slice"]
    return (nid // c["slice"]) * c["slice_pad"] + (nl % 128) * c["tb"] + nl // 128


def _acc_row(nl, c):
    return (nl % 128) * c["tb"] + nl // 128


def host_prep(x, edge_index, c):
    """Build per-core edge maps + the shared (max-over-cores) window schedule.

    Returns (edge_maps, sched): edge_maps[core] has gidx [16, ntot*8] i16
    (wrap16 gather indices) and dstoff [128, ntot] u8 (255 = pad slot).
    """
    n, cores = c["n"], c["cores"]
    sl, tb, cb = c["slice"], c["tb"], c["cb"]
    nwin = c["nwin"]
    src = np.concatenate([np.asarray(edge_index[0], np.int64),
                          np.arange(n, dtype=np.int64)])
    dst = np.concatenate([np.asarray(edge_index[1], np.int64),
                          np.arange(n, dtype=np.int64)])
    trow = _table_row(src, c)
    half = (trow >= c["half_rows"]).astype(np.int64)
    owner = dst // sl
    dloc = dst % sl
    win = dloc // WW

    # counts per (core, half, window) in one bincount
    key = (owner * 2 + half) * nwin + win
    counts = np.bincount(key, minlength=cores * 2 * nwin).reshape(cores, 2, nwin)
    tpw = -(-counts.max(axis=0) // 128)          # [2, nwin] tiles per window
    ntiles = tpw.sum(axis=1)
    for h in (0, 1):
        padt = (-int(ntiles[h])) % cb
        if padt:
            nz = np.nonzero(tpw[h])[0]
            wlast = int(nz[-1]) if len(nz) else 0
            tpw[h, wlast] += padt
            ntiles[h] += padt
    sched = dict(tpw=tpw, ntiles=[int(ntiles[0]), int(ntiles[1])])

    ntot = int(ntiles.sum())
    cap = ntot * 128
    # tile start (global, across both halves) for each (half, window)
    tstart = np.zeros((2, nwin), np.int64)
    tstart[0] = np.cumsum(tpw[0]) - tpw[0]
    tstart[1] = int(ntiles[0]) + np.cumsum(tpw[1]) - tpw[1]

    maps = []
    for core in range(cores):
        m = owner == core
        tr_c = trow[m]
        dl_c = dloc[m]
        hf_c = half[m]
        # order edges by (half, dloc) -> grouped per (half, window), windows
        # ascending, and dloc ascending inside each window
        order = np.lexsort((dl_c, hf_c))
        tr_c, dl_c, hf_c = tr_c[order], dl_c[order], hf_c[order]
        wn_c = dl_c // WW

        # slot of each edge: base of its (half, window) bucket + rank inside
        bkey = hf_c * nwin + wn_c
        bcnt = np.bincount(bkey, minlength=2 * nwin)
        bstart = np.cumsum(bcnt) - bcnt             # start idx in sorted order
        rank = np.arange(len(bkey)) - bstart[bkey]
        base = (tstart.reshape(-1)[bkey]) * 128
        slots = base + rank
        assert np.all(rank < tpw.reshape(-1)[bkey] * 128)

        srcrow = np.zeros(cap, np.int64)            # pads: row 0
        dstoff = np.full(cap, 255, np.uint8)        # pads: no one-hot match
        srcrow[slots] = tr_c - hf_c * c["half_rows"]
        dstoff[slots] = (dl_c % 128).astype(np.uint8)

        def wrap16(vals):
            nq = ntot // cb
            v = vals.reshape(nq, cb * 128)
            w16 = np.zeros((nq, 16, cb * 8), np.int16)
            k = np.arange(cb * 128)
            w16[:, k % 16, k // 16] = v
            return w16.transpose(1, 0, 2).reshape(16, nq * cb * 8)

        maps.append(dict(
            gidx=wrap16(srcrow.astype(np.int16)),
            dstoff=np.ascontiguousarray(dstoff.reshape(ntot, 128).T),
        ))
    return maps, sched


def host_pack(x, edge_maps, sched, W1, a_src1, a_dst1, b1, W2, a_src2,
              a_dst2, b2, Wout, bout, c):
    """Assemble the per-core [R, 128] int16 pack tensors."""
    heads, hid, hc, tb = c["heads"], c["hid"], c["hc"], c["tb"]
    sl, sp = c["slice"], c["slice_pad"]
    ntot = int(sched["ntiles"][0] + sched["ntiles"][1])
    lay = pack_layout(c, ntot)

    def blockdiag(a):
        A = np.zeros((hc, heads), np.float32)
        for h in range(heads):
            A[h * hid:(h + 1) * hid, h] = a[h]
        return A

    W1 = np.asarray(W1, np.float32)
    W2 = np.asarray(W2, np.float32)
    WA1 = np.concatenate([W1 @ blockdiag(np.asarray(a_src1, np.float32)),
                          W1 @ blockdiag(np.asarray(a_dst1, np.float32))], axis=1)
    WA2 = np.concatenate([W2 @ blockdiag(np.asarray(a_src2, np.float32)),
                          W2 @ blockdiag(np.asarray(a_dst2, np.float32))], axis=1)
    misc = np.zeros((128, 128), np.float16)
    misc[:, 0:16] = WA1.astype(np.float16)
    misc[:, 16:32] = WA2.astype(np.float16)
    misc[:, 32:42] = np.asarray(Wout, np.float32).astype(np.float16)
    rows4 = np.zeros((4, 128), np.float16)
    rows4[0] = np.arange(128, dtype=np.float16)
    rows4[1] = np.asarray(b1, np.float32).astype(np.float16)
    rows4[2] = np.asarray(b2, np.float32).astype(np.float16)
    rows4[3, 0:c["ncls"]] = np.asarray(bout, np.float32).astype(np.float16)

    x = np.asarray(x, np.float32)
    ngx = -(-tb // 2)
    packs = []
    for core in range(c["cores"]):
        em = edge_maps[core]
        p = np.zeros((lay["R"], 128), np.int16)
        xs = np.zeros((sp, c["in_ch"]), np.float32)
        xs[:sl] = x[core * sl:(core + 1) * sl]
        s = np.maximum(np.abs(xs).max(axis=1) / 127.0, 1e-4).astype(np.float16)
        xq = np.round(xs / s[:, None].astype(np.float32)).clip(-127, 127)
        # per-tile transposed lhsT blocks [128 in_ch, 128 nodes], int8
        xT = np.ascontiguousarray(
            xq.astype(np.int8).reshape(tb, 128, c["in_ch"]).transpose(0, 2, 1))
        blocks = np.zeros((ngx, 128, 256), np.int8)
        blocks[:, :, 0:128] = xT[0::2]
        blocks[: tb // 2, :, 128:256] = xT[1::2]
        p[lay["xT8"]:lay["xT8"] + ngx * 128] = \
            blocks.reshape(ngx * 128, 256).view(np.int16)
        srow = np.zeros((128, 128), np.float16)
        srow[:, :tb] = s.reshape(tb, 128).T
        p[lay["sS"]:lay["sS"] + 128] = srow.view(np.int16)

        K = lay["K"]
        gi = np.zeros((16, K * 128), np.int16)
        gi[:, :ntot * 8] = em["gidx"]
        p[lay["gidx"]:lay["gidx"] + 16 * K] = gi.reshape(16 * K, 128)

        Q = lay["Q"]
        doff = np.full((128, Q * 256), 255, np.uint8)
        doff[:, :ntot] = em["dstoff"]
        p[lay["dstoff"]:lay["dstoff"] + Q * 128] = np.ascontiguousarray(
            doff.reshape(128, Q, 256).transpose(1, 0, 2)
        ).reshape(Q * 128, 256).view(np.int16)

        p[lay["W1"]:lay["W1"] + 128] = W1.astype(np.float16).view(np.int16)
        p[lay["W2"]:lay["W2"] + 128] = W2.astype(np.float16).view(np.int16)
        p[lay["misc"]:lay["misc"] + 128] = misc.view(np.int16)
        p[lay["rows4"]:lay["rows4"] + 4] = rows4.view(np.int16)
        packs.append(p)
    return packs


def host_post(results, c):
    n = c["n"]
    out = np.zeros((n, c["ncls"]), np.float32)
    rows = _acc_row(np.arange(c["slice"]), c)
    for core in range(c["cores"]):
        res = np.asarray(results[core]["out"], np.float32)
        out[core * c["slice"]:(core + 1) * c["slice"]] = res[rows]
    return out


# ---------------------------------------------------------------- device build

def build_nc(c, sched, run_edges=True):
    from concourse import bass, mybir, bacc, tile
    from concourse.masks import make_identity

    f32 = mybir.dt.float32
    f16 = mybir.dt.float16
    i16 = mybir.dt.int16
    i8 = mybir.dt.int8
    u8 = mybir.dt.uint8
    Alu = mybir.AluOpType
    Act = mybir.ActivationFunctionType

    nc = bacc.Bacc("TRN2", target_bir_lowering=False, debug=False,
                   num_devices=c["cores"])
    cores = list(range(c["cores"]))

    tb, cb = c["tb"], c["cb"]
    hc, heads, ncls = c["hc"], c["heads"], c["ncls"]
    trw, mw = c["trw"], c["mw"]
    sp, nwin = c["slice_pad"], c["nwin"]
    tpw, ntiles = sched["tpw"], sched["ntiles"]
    ntot = int(ntiles[0] + ntiles[1])
    lay = pack_layout(c, ntot)
    K, Q = lay["K"], lay["Q"]

    # ---- I/O
    pack = nc.dram_tensor("pack", [lay["R"], 128], i16, kind="ExternalInput")
    out = nc.dram_tensor("out", [sp, ncls], f16, kind="ExternalOutput")

    # ---- internal DRAM
    bounce1 = nc.dram_tensor("bounce1", [sp, trw], f32)
    bounce2 = nc.dram_tensor("bounce2", [sp, trw], f32)
    tspace = "Shared" if c["cores"] > 4 else "Local"
    table1 = nc.dram_tensor("table1", [c["table_rows"], trw], f32, addr_space=tspace)
    table2 = nc.dram_tensor("table2", [c["table_rows"], trw], f32, addr_space=tspace)

    with tile.TileContext(nc) as tc:
        with (
            tc.tile_pool(name="const", bufs=1) as constp,
            tc.tile_pool(name="rec", bufs=1) as recp,
            tc.tile_pool(name="big", bufs=2) as bigp,
            tc.tile_pool(name="accs", bufs=1) as accsp,
            tc.tile_pool(name="small", bufs=2) as smallp,
            tc.tile_pool(name="work", bufs=2) as workp,
            tc.tile_pool(name="oh", bufs=2 * c["cb"]) as ohp,
            tc.tile_pool(name="oht", bufs=3) as ohtp,
            tc.tile_pool(name="psA", bufs=1, space="PSUM") as psA,
            tc.tile_pool(name="psB", bufs=1, space="PSUM") as psB,
            tc.tile_pool(name="psD", bufs=1, space="PSUM") as psD,
            tc.tile_pool(name="psW", bufs=2, space="PSUM") as psW,
            tc.tile_pool(name="psT2", bufs=2, space="PSUM") as psT2,
            tc.tile_pool(name="psAD", bufs=1, space="PSUM") as psAD,
        ):
            # ---------------- constants ----------------
            ident = constp.tile([128, 128], f32, tag="ident")
            make_identity(nc, ident[:])

            W1s = constp.tile([128, hc], f16, tag="W1s")
            nc.sync.dma_start(W1s[:], pack[lay["W1"]:lay["W1"] + 128, :].bitcast(f16))
            W2s = constp.tile([128, hc], f16, tag="W2s")
            nc.sync.dma_start(W2s[:], pack[lay["W2"]:lay["W2"] + 128, :].bitcast(f16))
            miscS = constp.tile([128, 128], f16, tag="miscS")
            nc.sync.dma_start(miscS[:], pack[lay["misc"]:lay["misc"] + 128, :].bitcast(f16))
            rowsS = []
            for i in range(4):
                rS = constp.tile([1, 128], f16, tag=f"row{i}S")
                nc.sync.dma_start(
                    rS[:], pack[lay["rows4"] + i:lay["rows4"] + i + 1, :].bitcast(f16))
                rowsS.append(rS)
            onesr = constp.tile([1, 128], f16, tag="onesr")
            nc.vector.memset(onesr[:], 1.0)

            # broadcast single rows across partitions: rank-1 PE matmul
            def bcast_row(row_ap, width, tag):
                ps = psA.tile([128, 128], f32, tag="psT")
                nc.tensor.matmul(out=ps[:, 0:width], lhsT=onesr[:],
                                 rhs=row_ap, start=True, stop=True)
                t = constp.tile([128, width], f32, tag=tag)
                nc.any.tensor_copy(out=t[:], in_=ps[:, 0:width])
                return t

            iotaS = bcast_row(rowsS[0][:], 128, "iotaS")
            b1s = bcast_row(rowsS[1][:], hc, "b1s")
            b2s = bcast_row(rowsS[2][:], hc, "b2s")
            bouts = bcast_row(rowsS[3][:, 0:ncls], ncls, "bouts")

            # gather index tables: load [16, K*128] and replicate to 128 parts
            gidxS = constp.tile([128, K * 128], i16, tag="gidxS")
            gsrc = pack[lay["gidx"]:lay["gidx"] + 16 * K, :].rearrange(
                "(p k) w -> p (k w)", p=16)
            for k in range(8):
                nc.sync.dma_start(gidxS[16 * k:16 * (k + 1), :], gsrc)

            dstoff8 = constp.tile([128, Q * 256], u8, tag="dstoff8")
            for q in range(Q):
                nc.sync.dma_start(
                    dstoff8[:, q * 256:(q + 1) * 256],
                    pack[lay["dstoff"] + q * 128:lay["dstoff"] + (q + 1) * 128, :]
                    .bitcast(u8))
            dstoffS = constp.tile([128, ntot], f32, tag="dstoffS")
            nc.vector.tensor_copy(out=dstoffS[:], in_=dstoff8[:, 0:ntot])

            sS16 = constp.tile([128, 128], f16, tag="sS16")
            nc.sync.dma_start(sS16[:], pack[lay["sS"]:lay["sS"] + 128, :].bitcast(f16))
            sF = constp.tile([128, tb], f32, tag="sF")
            nc.vector.tensor_copy(out=sF[:], in_=sS16[:, 0:tb])

            accS = accsp.tile([128, tb, mw], f32, tag="accS")

            # ---------------- record-slice build ----------------
            def build_records(get_lhsT, W, WA, rec, scale=None):
                nc.vector.memset(rec[:], 0.0)
                for t in range(tb):
                    lt = get_lhsT(t)
                    h_p = psB.tile([128, hc], f32, tag="psH")
                    nc.tensor.matmul(out=h_p[:], lhsT=lt, rhs=W, start=True, stop=True)
                    a_p = psD.tile([128, 2 * heads], f32, tag="psAS")
                    nc.tensor.matmul(out=a_p[:], lhsT=lt, rhs=WA, start=True, stop=True)
                    if scale is None:
                        nc.any.tensor_copy(out=rec[:, t, 0:hc], in_=h_p[:])
                        nc.any.tensor_copy(out=rec[:, t, hc:hc + 2 * heads], in_=a_p[:])
                    else:
                        nc.vector.tensor_scalar(
                            out=rec[:, t, 0:hc], in0=h_p[:],
                            scalar1=scale[:, t:t + 1], scalar2=None, op0=Alu.mult)
                        nc.vector.tensor_scalar(
                            out=rec[:, t, hc:hc + 2 * heads], in0=a_p[:],
                            scalar1=scale[:, t:t + 1], scalar2=None, op0=Alu.mult)

            def publish(rec, bounce, table):
                nc.sync.dma_start(
                    bounce[:].rearrange("(p t) w -> p t w", p=128), rec[:]
                )
                nc.gpsimd.collective_compute(
                    "AllGather", mybir.AluOpType.bypass,
                    replica_groups=[cores], ins=[bounce[:]], outs=[table[:]],
                )

            # ---------------- edge phase ----------------
            def edge_phase(table, rec):
                nc.vector.memset(accS[:], 0.0)
                if not run_edges:
                    return
                tile_base = 0
                for h in (0, 1):
                    tab_h = table[h * c["half_rows"]:(h + 1) * c["half_rows"], :]
                    nt_h = int(ntiles[h])
                    nq = nt_h // cb
                    wins = []
                    twin = []                     # tile (within half) -> window
                    t0 = 0
                    for w in range(nwin):
                        tcnt = int(tpw[h, w])
                        if tcnt:
                            wins.append((w, t0, tcnt))
                            twin.extend([w] * tcnt)
                            t0 += tcnt
                    assert t0 == nt_h
                    widx = 0
                    psw = None
                    for q in range(nq):
                        grec = bigp.tile([128, cb, trw], f32, tag="grec")
                        ccol = (tile_base + q * cb) * 8
                        nc.gpsimd.dma_gather(
                            out_ap=grec[:], in_ap=tab_h,
                            idxs_ap=gidxS[:, ccol:ccol + cb * 8],
                            num_idxs=cb * 128, num_idxs_reg=cb * 128,
                            elem_size=trw,
                        )
                        # one-hots for the chunk's tiles + per-edge a_dst via
                        # transposed-one-hot matmul against the local records
                        ohs = []
                        ps_ad = psAD.tile([128, cb, heads], f32, tag="psad")
                        for b in range(cb):
                            g_h = q * cb + b
                            gg = tile_base + g_h
                            oh = ohp.tile([128, 128], f32, tag="oh")
                            nc.vector.tensor_scalar(
                                out=oh[:], in0=iotaS[:],
                                scalar1=dstoffS[:, gg:gg + 1], scalar2=None,
                                op0=Alu.is_equal,
                            )
                            ohs.append(oh)
                            ohT_p = psT2.tile([128, 128], f32, tag="psoT")
                            nc.tensor.transpose(out=ohT_p[:], in_=oh[:],
                                                identity=ident[:])
                            ohTs = ohtp.tile([128, 128], f32, tag="ohT")
                            nc.any.tensor_copy(out=ohTs[:], in_=ohT_p[:])
                            wb = twin[g_h]
                            nc.tensor.matmul(
                                out=ps_ad[:, b, :], lhsT=ohTs[:],
                                rhs=rec[:, wb, hc + heads:hc + 2 * heads],
                                start=True, stop=True,
                            )
                        wv = smallp.tile([128, cb, heads], f32, tag="wv")
                        tmp = smallp.tile([128, cb, heads], f32, tag="tmp")
                        nc.vector.tensor_tensor(
                            out=wv[:], in0=grec[:, :, hc:hc + heads],
                            in1=ps_ad[:], op=Alu.add,
                        )
                        nc.vector.tensor_scalar(
                            out=tmp[:], in0=wv[:], scalar1=0.0,
                            scalar2=-(1.0 - NEG_SLOPE), op0=Alu.min, op1=Alu.mult,
                        )
                        nc.vector.tensor_tensor(
                            out=wv[:], in0=wv[:], in1=tmp[:], op=Alu.add,
                        )
                        nc.scalar.activation(out=wv[:], in_=wv[:], func=Act.Exp)
                        nc.vector.tensor_tensor(
                            out=grec[:, :, 0:hc].rearrange(
                                "p b (h d) -> p b h d", h=heads),
                            in0=grec[:, :, 0:hc].rearrange(
                                "p b (h d) -> p b h d", h=heads),
                            in1=wv[:].unsqueeze(-1).to_broadcast(
                                [128, cb, heads, c["hid"]]),
                            op=Alu.mult,
                        )
                        nc.vector.tensor_copy(
                            out=grec[:, :, hc:hc + heads], in_=wv[:]
                        )
                        for b in range(cb):
                            g_h = q * cb + b
                            w, t0w, tcnt = wins[widx]
                            if g_h == t0w:
                                psw = psW.tile([128, mw], f32, tag="psw")
                            nc.tensor.matmul(
                                out=psw[:], lhsT=ohs[b][:], rhs=grec[:, b, 0:mw],
                                start=g_h == t0w, stop=g_h == t0w + tcnt - 1,
                            )
                            if g_h == t0w + tcnt - 1:
                                nc.vector.tensor_tensor(
                                    out=accS[:, w, :], in0=accS[:, w, :],
                                    in1=psw[:], op=Alu.add,
                                )
                                widx += 1
                    tile_base += nt_h

            # ---------------- divide + bias + relu ----------------
            def finish_layer(bias, ytile):
                rcp = smallp.tile([128, tb, heads], f32, tag="rcp")
                nc.vector.tensor_scalar(
                    out=rcp[:], in0=accS[:, :, hc:hc + heads],
                    scalar1=1e-9, scalar2=None, op0=Alu.add,
                )
                nc.vector.reciprocal(out=rcp[:], in_=rcp[:])
                nc.vector.tensor_tensor(
                    out=ytile[:].rearrange("p t (h d) -> p t h d", h=heads),
                    in0=accS[:, :, 0:hc].rearrange("p t (h d) -> p t h d", h=heads),
                    in1=rcp[:].unsqueeze(-1).to_broadcast([128, tb, heads, c["hid"]]),
                    op=Alu.mult,
                )
                nc.vector.tensor_tensor(
                    out=ytile[:], in0=ytile[:],
                    in1=bias.unsqueeze(1).to_broadcast([128, tb, hc]),
                    op=Alu.add,
                )
                nc.vector.tensor_scalar(
                    out=ytile[:], in0=ytile[:], scalar1=0.0, scalar2=None,
                    op0=Alu.max,
                )

            # lhsT providers: layer 1 reads pre-transposed f16 tiles from the
            # pack; later layers transpose on-device and downcast to f16.
            def x_lhsT(t):
                g, hb = t // 2, t % 2
                xq8 = workp.tile([128, 128], i8, tag="xq8")
                r0 = lay["xT8"] + g * 128
                nc.sync.dma_start(
                    xq8[:], pack[r0:r0 + 128, 64 * hb:64 * (hb + 1)].bitcast(i8))
                xt = workp.tile([128, 128], f16, tag="xt")
                nc.vector.tensor_copy(out=xt[:], in_=xq8[:])
                return xt[:]

            def make_y_lhsT(ytile):
                def y_lhsT(t):
                    yT_p = psA.tile([128, 128], f32, tag="psT")
                    nc.tensor.transpose(out=yT_p[:], in_=ytile[:, t, :],
                                        identity=ident[:])
                    yTs = workp.tile([128, 128], f16, tag="xt")
                    nc.any.tensor_copy(out=yTs[:], in_=yT_p[:])
                    return yTs[:]
                return y_lhsT

            # ================ layer 1 ================
            rec1 = recp.tile([128, tb, trw], f32, tag="rec")
            build_records(x_lhsT, W1s[:], miscS[:, 0:16], rec1, scale=sF)
            publish(rec1, bounce1, table1)
            edge_phase(table1, rec1)
            y1 = recp.tile([128, tb, hc], f32, tag="y")
            finish_layer(b1s[:], y1)

            # ================ layer 2 ================
            rec2 = recp.tile([128, tb, trw], f32, tag="rec")
            build_records(make_y_lhsT(y1), W2s[:], miscS[:, 16:32], rec2)
            publish(rec2, bounce2, table2)
            edge_phase(table2, rec2)
            y2 = recp.tile([128, tb, hc], f32, tag="y")
            finish_layer(b2s[:], y2)

            # ================ output projection ================
            outt = recp.tile([128, tb, ncls], f16, tag="outt")
            y_lhsT2 = make_y_lhsT(y2)
            for t in range(tb):
                yTs = y_lhsT2(t)
                o_p = psD.tile([128, 2 * heads], f32, tag="psAS")
                nc.tensor.matmul(out=o_p[:, 0:ncls], lhsT=yTs,
                                 rhs=miscS[:, 32:32 + ncls],
                                 start=True, stop=True)
                nc.vector.tensor_tensor(
                    out=outt[:, t, :], in0=o_p[:, 0:ncls], in1=bouts[:],
                    op=Alu.add,
                )
            nc.sync.dma_start(
                out[:].rearrange("(p t) w -> p t w", p=128), outt[:]
            )

    nc.compile()
    return nc


# ---------------------------------------------------------------- entry point

_CACHE = {}


def prepare(inputs, c):
    """inputs dict -> (in_maps, sched)."""
    x = np.asarray(inputs["x"], np.float32)
    edge_index = np.asarray(inputs["edge_index"])
    edge_maps, sched = host_prep(x, edge_index, c)
    packs = host_pack(
        x, edge_maps, sched, inputs["W1"], inputs["a_src1"], inputs["a_dst1"],
        inputs["b1"], inputs["W2"], inputs["a_src2"], inputs["a_dst2"],
        inputs["b2"], inputs["Wout"], inputs["bout"], c)
    in_maps = [dict(pack=p) for p in packs]
    return in_maps, sched


def kernel(x, edge_index, W1, a_src1, a_dst1, b1, W2, a_src2, a_dst2, b2,
           Wout, bout):
    from concourse.bass_utils import run_bass_kernel_spmd

    c = derive(full_cfg())
    in_maps, sched = prepare(dict(
        x=x, edge_index=edge_index, W1=W1, a_src1=a_src1, a_dst1=a_dst1,
        b1=b1, W2=W2, a_src2=a_src2, a_dst2=a_dst2, b2=b2, Wout=Wout,
        bout=bout), c)
    key = ("full", sched["tpw"].tobytes())
    if key not in _CACHE:
        _CACHE[key] = build_nc(c, sched)
    nc = _CACHE[key]
    res = run_bass_kernel_spmd(nc, in_maps, list(range(c["cores"])))
    return host_post(res.results, c)
